# revision 29
# baseline (speedup 1.0000x reference)
"""Linear-attention kernel (out = (relu(Q)+eps) @ ((relu(K)+eps)^T V)) on 8 TRN2 cores.

Sharding: data-parallel over batch B=8 -> one batch per NeuronCore, no comm.
Per core: S=4096, D=256, DV=256, fp32 out.

Numerics: Q/K/V are cast to fp16 on the host (halves HBM->SBUF traffic; the
rounding point is identical to casting on-device). All matmul operands fp16,
PSUM accumulation fp32, output stored fp32.
"""

from contextlib import ExitStack

import numpy as np

import concourse.bacc as bacc
import concourse.bass as bass
import concourse.mybir as mybir
from concourse.bass_utils import run_bass_kernel_spmd
from concourse.masks import make_identity
from concourse.tile import TileContext

B, S, D, DV = 8, 4096, 256, 256
P = 128
NCH = S // P            # 32 chunks of 128 sequence rows
GRP = 8                 # chunks per DMA piece (512 KiB fp16)
NGRP = NCH // GRP       # 4
EPS = 1e-6
F32 = mybir.dt.float32
F16 = mybir.dt.float16
MAX = mybir.AluOpType.max
ADD = mybir.AluOpType.add

_CACHE: dict = {}


def _build() -> bass.Bass:
    nc = bacc.Bacc("TRN2", target_bir_lowering=False)
    Kd = nc.declare_dram_parameter("K", [S, D], F16, isOutput=False)
    Vd = nc.declare_dram_parameter("V", [S, DV], F16, isOutput=False)
    Qd = nc.declare_dram_parameter("Q", [S, D], F16, isOutput=False)
    Od = nc.declare_dram_parameter("out", [S, DV], F32, isOutput=True)

    # seq row index s = p*NCH + n: partition-major so each partition's DMA
    # span is contiguous in DRAM.
    Kv = Kd[:, :].rearrange("(p n) d -> p n d", p=P)
    Vv = Vd[:, :].rearrange("(p n) d -> p n d", p=P)
    Qv = Qd[:, :].rearrange("(p n) d -> p n d", p=P)
    Ov = Od[:, :].rearrange("(p n) d -> p n d", p=P)

    with TileContext(nc) as tc, ExitStack() as ctx:
        consts = ctx.enter_context(tc.tile_pool(name="consts", bufs=1))
        big = ctx.enter_context(tc.tile_pool(name="big", bufs=1))
        pkv = ctx.enter_context(tc.tile_pool(name="pkv", bufs=1, space="PSUM"))
        pqt = ctx.enter_context(tc.tile_pool(name="pqt", bufs=2, space="PSUM"))
        pout = ctx.enter_context(tc.tile_pool(name="pout", bufs=4, space="PSUM"))

        ident = consts.tile([P, P], F16, name="ident")
        make_identity(nc, ident)

        # Per-piece staging tiles (one DMA writer each).
        kts = [big.tile([P, GRP, D], F16, name=f"kt{g}") for g in range(NGRP)]
        vts = [big.tile([P, GRP, DV], F16, name=f"vt{g}") for g in range(NGRP)]
        qts = [big.tile([P, GRP, D], F16, name=f"qt{g}") for g in range(NGRP)]
        qtT = big.tile([P, NCH, D], F16, name="qtT")   # (relu(Q)+eps)^T tiles
        ot = big.tile([P, NCH, DV], F32, name="ot")    # output staging
        kv = big.tile([P, 2, DV], F16, name="kv")      # KV = K_^T V, d-halves

        # Loads (HWDGE on Sync): K/V first at full bandwidth -- the critical
        # chain is K/V -> phase 1 -> KV -> phase 2. Q pieces trail; the
        # transposes and phase-2 matmuls they gate are cheap and pipeline
        # into the tail.
        # Two HWDGE rings (Sync + Scalar) in parallel: each ring's transfers
        # are FIFO-serial, so alternating pieces across rings roughly doubles
        # in-flight load bandwidth.
        rings = [nc.sync, nc.scalar]
        pieces = [(kts[g], Kv, g) for g in range(NGRP)]
        pieces += [(vts[g], Vv, g) for g in range(NGRP)]
        order = [0, NGRP, 1, NGRP + 1, 2, NGRP + 2, 3, NGRP + 3]
        for i, idx in enumerate(order):
            tile_, view, g = pieces[idx]
            s = slice(g * GRP, (g + 1) * GRP)
            rings[i % 2].dma_start(out=tile_[:, :, :], in_=view[:, s, :])
        for g in range(NGRP):
            s = slice(g * GRP, (g + 1) * GRP)
            rings[g % 2].dma_start(out=qts[g][:, :, :], in_=Qv[:, s, :])

        # All relus up front in DVE program order (arrival order: K's then
        # Q's); they gate the PE stream and copybacks queue behind them.
        for g in range(NGRP):
            nc.vector.tensor_scalar(
                out=kts[g][:, :, :], in0=kts[g][:, :, :],
                scalar1=0.0, scalar2=EPS, op0=MAX, op1=ADD,
            )
        for g in range(NGRP):
            nc.vector.tensor_scalar(
                out=qts[g][:, :, :], in0=qts[g][:, :, :],
                scalar1=0.0, scalar2=EPS, op0=MAX, op1=ADD,
            )

        kvps = [pkv.tile([P, DV], F32, name=f"kvps{h}") for h in range(2)]

        # Phase 1 back-to-back on the PE: KV[d, v] += K_[k, d] * V[k, v].
        for g in range(NGRP):
            for j in range(GRP):
                n = g * GRP + j
                for h in range(2):
                    nc.tensor.matmul(
                        kvps[h][:, :],
                        kts[g][:, j, h * P:(h + 1) * P],
                        vts[g][:, j, :],
                        start=(n == 0), stop=(n == NCH - 1),
                    )
        for h in range(2):
            nc.scalar.copy(kv[:, h, :], kvps[h][:, :])

        # Tail: per Q piece, transpose its tiles on the PE (8 transposes =
        # 4 chunks x 2 halves batched into one PSUM bank + one wide copyback),
        # then immediately run those chunks' phase-2 matmuls.
        for g in range(NGRP):
            for q4 in range(GRP // 4):
                ps_t = pqt.tile([P, 8, P], F16, name="ps_t")
                for i2 in range(4):
                    j = q4 * 4 + i2
                    for h in range(2):
                        nc.tensor.transpose(
                            ps_t[:, i2 * 2 + h, :],
                            qts[g][:, j, h * P:(h + 1) * P], ident,
                        )
                n0 = g * GRP + q4 * 4
                dst = qtT[:, n0:n0 + 4, :]
                if (g * 2 + q4) % 2 == 0:
                    nc.vector.tensor_copy(dst, ps_t[:, :, :])
                else:
                    nc.scalar.copy(dst, ps_t[:, :, :])
            # Phase 2 for this piece's 8 chunks, two chunks per PSUM bank.
            for n2 in range(GRP // 2):
                ps_o = pout.tile([P, 2, DV], F32, name="ps_o")
                for i2 in range(2):
                    n = g * GRP + n2 * 2 + i2
                    for h in range(2):
                        nc.tensor.matmul(
                            ps_o[:, i2, :],
                            qtT[:, n, h * P:(h + 1) * P],
                            kv[:, h, :],
                            start=(h == 0), stop=(h == 1),
                        )
                n0 = g * GRP + n2 * 2
                dst = ot[:, n0:n0 + 2, :]
                if n2 % 2 == 0:
                    nc.vector.tensor_copy(dst, ps_o[:, :, :])
                else:
                    nc.scalar.copy(dst, ps_o[:, :, :])
                if n2 % 2 == 1:
                    g4 = (g * GRP + n2 * 2) // 4
                    s = slice(g4 * 4, (g4 + 1) * 4)
                    # Stores ride the Sync ring, idle once loads are done.
                    nc.sync.dma_start(out=Ov[:, s, :], in_=ot[:, s, :])

    nc.compile()
    return nc


def _run(Q, K, V, trace=False, **trace_kwargs):
    if "nc" not in _CACHE:
        _CACHE["nc"] = _build()
    nc = _CACHE["nc"]
    Q = np.asarray(Q, dtype=np.float32).astype(np.float16)
    K = np.asarray(K, dtype=np.float32).astype(np.float16)
    V = np.asarray(V, dtype=np.float32).astype(np.float16)
    in_maps = [{"Q": Q[b], "K": K[b], "V": V[b]} for b in range(B)]
    res = run_bass_kernel_spmd(
        nc, in_maps, core_ids=list(range(B)), trace=trace, **trace_kwargs
    )
    out = np.stack([res.results[b]["out"] for b in range(B)], axis=0)
    return out, res


def kernel(Q, K, V):
    out, _ = _run(Q, K, V, trace=False)
    return out


# revision 30
# speedup vs baseline: 1.1194x; 1.1194x over previous
"""Linear-attention kernel (out = (relu(Q)+eps) @ ((relu(K)+eps)^T V)) on 8 TRN2 cores.

Sharding: data-parallel over batch B=8 -> one batch per NeuronCore, no comm.
Per core: S=4096, D=256, DV=256, fp32 out.

Numerics: Q/K/V are cast to fp16 on the host (halves HBM->SBUF traffic; the
rounding point is identical to casting on-device). All matmul operands fp16,
PSUM accumulation fp32, output stored fp32.
"""

from contextlib import ExitStack

import numpy as np

import concourse.bacc as bacc
import concourse.bass as bass
import concourse.mybir as mybir
from concourse.bass_utils import run_bass_kernel_spmd
from concourse.masks import make_identity
from concourse.tile import TileContext

B, S, D, DV = 8, 4096, 256, 256
P = 128
NCH = S // P            # 32 chunks of 128 sequence rows
GRP = 8                 # chunks per DMA piece (512 KiB fp16)
NGRP = NCH // GRP       # 4
EPS = 1e-6
F32 = mybir.dt.float32
F16 = mybir.dt.float16
MAX = mybir.AluOpType.max
ADD = mybir.AluOpType.add

_CACHE: dict = {}


def _build() -> bass.Bass:
    nc = bacc.Bacc("TRN2", target_bir_lowering=False)
    Kd = nc.declare_dram_parameter("K", [S, D], F16, isOutput=False)
    Vd = nc.declare_dram_parameter("V", [S, DV], F16, isOutput=False)
    Qd = nc.declare_dram_parameter("Q", [S, D], F16, isOutput=False)
    Od = nc.declare_dram_parameter("out", [S, DV], F32, isOutput=True)

    # seq row index s = p*NCH + n: partition-major so each partition's DMA
    # span is contiguous in DRAM.
    Kv = Kd[:, :].rearrange("(p n) d -> p n d", p=P)
    Vv = Vd[:, :].rearrange("(p n) d -> p n d", p=P)
    Qv = Qd[:, :].rearrange("(p n) d -> p n d", p=P)
    Ov = Od[:, :].rearrange("(p n) d -> p n d", p=P)

    with TileContext(nc) as tc, ExitStack() as ctx:
        consts = ctx.enter_context(tc.tile_pool(name="consts", bufs=1))
        big = ctx.enter_context(tc.tile_pool(name="big", bufs=1))
        pkv = ctx.enter_context(tc.tile_pool(name="pkv", bufs=1, space="PSUM"))
        pqt = ctx.enter_context(tc.tile_pool(name="pqt", bufs=2, space="PSUM"))
        pout = ctx.enter_context(tc.tile_pool(name="pout", bufs=4, space="PSUM"))

        ident = consts.tile([P, P], F16, name="ident")
        make_identity(nc, ident)

        # Per-piece staging tiles (one DMA writer each).
        kts = [big.tile([P, GRP, D], F16, name=f"kt{g}") for g in range(NGRP)]
        vts = [big.tile([P, GRP, DV], F16, name=f"vt{g}") for g in range(NGRP)]
        qts = [big.tile([P, GRP, D], F16, name=f"qt{g}") for g in range(NGRP)]
        qtT = big.tile([P, NCH, D], F16, name="qtT")   # (relu(Q)+eps)^T tiles
        ot = big.tile([P, NCH, DV], F32, name="ot")    # output staging
        kv = big.tile([P, 2, DV], F16, name="kv")      # KV = K_^T V, d-halves

        # Loads (HWDGE on Sync): K/V first at full bandwidth -- the critical
        # chain is K/V -> phase 1 -> KV -> phase 2. Q pieces trail; the
        # transposes and phase-2 matmuls they gate are cheap and pipeline
        # into the tail.
        for g in range(NGRP):
            s = slice(g * GRP, (g + 1) * GRP)
            nc.sync.dma_start(out=kts[g][:, :, :], in_=Kv[:, s, :])
            nc.sync.dma_start(out=vts[g][:, :, :], in_=Vv[:, s, :])
        for g in range(NGRP):
            s = slice(g * GRP, (g + 1) * GRP)
            nc.sync.dma_start(out=qts[g][:, :, :], in_=Qv[:, s, :])

        # All relus up front in DVE program order (arrival order: K's then
        # Q's); they gate the PE stream and copybacks queue behind them.
        for g in range(NGRP):
            nc.vector.tensor_scalar(
                out=kts[g][:, :, :], in0=kts[g][:, :, :],
                scalar1=0.0, scalar2=EPS, op0=MAX, op1=ADD,
            )
        for g in range(NGRP):
            nc.vector.tensor_scalar(
                out=qts[g][:, :, :], in0=qts[g][:, :, :],
                scalar1=0.0, scalar2=EPS, op0=MAX, op1=ADD,
            )

        kvps = [pkv.tile([P, DV], F32, name=f"kvps{h}") for h in range(2)]

        # Phase 1 back-to-back on the PE: KV[d, v] += K_[k, d] * V[k, v].
        for g in range(NGRP):
            for j in range(GRP):
                n = g * GRP + j
                for h in range(2):
                    nc.tensor.matmul(
                        kvps[h][:, :],
                        kts[g][:, j, h * P:(h + 1) * P],
                        vts[g][:, j, :],
                        start=(n == 0), stop=(n == NCH - 1),
                    )
        for h in range(2):
            nc.scalar.copy(kv[:, h, :], kvps[h][:, :])

        # Tail: per Q piece, transpose its tiles on the PE (8 transposes =
        # 4 chunks x 2 halves batched into one PSUM bank + one wide copyback),
        # then immediately run those chunks' phase-2 matmuls.
        for g in range(NGRP):
            for q4 in range(GRP // 4):
                ps_t = pqt.tile([P, 8, P], F16, name="ps_t")
                for i2 in range(4):
                    j = q4 * 4 + i2
                    for h in range(2):
                        nc.tensor.transpose(
                            ps_t[:, i2 * 2 + h, :],
                            qts[g][:, j, h * P:(h + 1) * P], ident,
                        )
                n0 = g * GRP + q4 * 4
                dst = qtT[:, n0:n0 + 4, :]
                if (g * 2 + q4) % 2 == 0:
                    nc.vector.tensor_copy(dst, ps_t[:, :, :])
                else:
                    nc.scalar.copy(dst, ps_t[:, :, :])
            # Phase 2 for this piece's 8 chunks, two chunks per PSUM bank.
            for n2 in range(GRP // 2):
                ps_o = pout.tile([P, 2, DV], F32, name="ps_o")
                for i2 in range(2):
                    n = g * GRP + n2 * 2 + i2
                    for h in range(2):
                        nc.tensor.matmul(
                            ps_o[:, i2, :],
                            qtT[:, n, h * P:(h + 1) * P],
                            kv[:, h, :],
                            start=(h == 0), stop=(h == 1),
                        )
                n0 = g * GRP + n2 * 2
                dst = ot[:, n0:n0 + 2, :]
                if n2 % 2 == 0:
                    nc.vector.tensor_copy(dst, ps_o[:, :, :])
                else:
                    nc.scalar.copy(dst, ps_o[:, :, :])
                if n2 % 2 == 1:
                    g4 = (g * GRP + n2 * 2) // 4
                    s = slice(g4 * 4, (g4 + 1) * 4)
                    # Stores ride the Sync ring, idle once loads are done.
                    nc.sync.dma_start(out=Ov[:, s, :], in_=ot[:, s, :])

    nc.compile()
    return nc


def _run(Q, K, V, trace=False, **trace_kwargs):
    if "nc" not in _CACHE:
        _CACHE["nc"] = _build()
    nc = _CACHE["nc"]
    Q = np.asarray(Q, dtype=np.float32).astype(np.float16)
    K = np.asarray(K, dtype=np.float32).astype(np.float16)
    V = np.asarray(V, dtype=np.float32).astype(np.float16)
    in_maps = [{"Q": Q[b], "K": K[b], "V": V[b]} for b in range(B)]
    res = run_bass_kernel_spmd(
        nc, in_maps, core_ids=list(range(B)), trace=trace, **trace_kwargs
    )
    out = np.stack([res.results[b]["out"] for b in range(B)], axis=0)
    return out, res


def kernel(Q, K, V):
    out, _ = _run(Q, K, V, trace=False)
    return out
